# revision 65
# baseline (speedup 1.0000x reference)
"""Tensor-parallel MultiHeadAttention kernel for 8 Trainium2 NeuronCores.

Problem (hardcoded): B=2, N=2048, C=1024, H=16 heads, D=64.
Sharding: core c handles batch b = c//4 and head group hg = c%4
(heads 4*hg .. 4*hg+3).  Each core computes Q/K/V for its 4 heads,
full attention for those heads, and a partial output projection; the
host sums the 4 partials per batch and adds the output bias.

V2 design (ScalarE-exp-bound schedule):
 - ScalarE runs ONLY the softmax exponentials ([128,1024] tiles); all
   PSUM->SBUF copies / bias adds run on DVE.
 - Scores are produced transposed (k-tokens on partitions); the AV
   matmul is "flipped": lhsT = probs block [128k,128q], rhs = V block
   [128k,65] -> out [128q,65].  Column 64 of V is constant 1.0 so the
   same accumulation also yields the softmax denominator per q row.
 - Normalization is a per-partition tensor_scalar multiply on DVE
   into head-pair [128,128] tiles; a DMA XBAR transpose brings O back
   feature-major for the output projection (no PE/PSUM cost).
 - The emission schedule interleaves the QKV projection (micro-items
   accumulating in the out-projection PSUM banks, which are free until
   the first out-projection), transposes and output projection into
   the slack of the scores/exp/AV unit stream, paced by a budget pump
   with correctness deadlines, so the exp chain starts early and never
   starves.
"""

import sys

import numpy as np
import ml_dtypes

try:
    import concourse.bass  # noqa: F401
except ImportError:  # fallback if PYTHONPATH lacks the repo
    for p in ("/opt/trn_rl_repo", "/root/.axon_site/_ro/trn_rl_repo"):
        if p not in sys.path:
            sys.path.insert(0, p)

B, N, C, H, D = 2, 2048, 1024, 16, 64
NCORES = 8
HPC = 4            # heads per core
DL = HPC * D       # 256 local feature dim
NB = N // 128      # 16 k-token blocks
DLV = HPC * (D + 1)  # 260 V columns per token block (64 feats + 1.0)

_cache: dict = {}


def _patch_drain_cap():
    """The walrus build in this container rejects instructions carrying
    more than a couple of sync-wait commands.  Split excess waits onto
    same-engine NoOps emitted just before the offending instruction."""
    import concourse.mybir as mybir
    from concourse.tile import TileContext
    from concourse.vector_clock import ScopedClock

    if getattr(TileContext, "_drain_cap_patched", False):
        return
    CAP = 1

    orig_commit = TileContext._commit_instruction

    def commit_split(self, inst, lazy_reg_writes=True):
        si = getattr(inst, "sync_info", None)
        if si is not None and si.on_wait is not None and len(si.on_wait) > CAP:
            waits = list(si.on_wait)
            keep = waits[len(waits) - CAP:]
            extra = waits[:len(waits) - CAP]
            for i in range(0, len(extra), CAP):
                nop = mybir.InstNoOp(
                    name=self.nc.get_next_instruction_name(),
                    engine=inst.engine,
                    sync_info=mybir.SyncInfo(on_wait=extra[i:i + CAP],
                                             on_update=[]),
                    bass_nofuse=True,
                )
                orig_commit(self, nop, lazy_reg_writes)
            inst.sync_info = mybir.SyncInfo(
                on_wait=keep, on_update=list(si.on_update))
        return orig_commit(self, inst, lazy_reg_writes)

    TileContext._commit_instruction = commit_split

    def patched(self, tick_clock, wait_clock):
        nc = self.nc
        drain_inst = nc.sync.drain()
        wait_clock.add_sem_waits(
            drain_inst.ins, ScopedClock({None: tick_clock.global_clock})
        )
        si = drain_inst.ins.sync_info
        if si is not None and len(si.on_wait) > CAP:
            waits = list(si.on_wait)
            drain_inst.ins.sync_info = mybir.SyncInfo(
                on_wait=waits[:CAP], on_update=list(si.on_update)
            )
            for i in range(CAP, len(waits), CAP):
                nop_bi = nc.sync.nop(nofuse=True)
                nop_bi.ins.sync_info = mybir.SyncInfo(
                    on_wait=waits[i : i + CAP], on_update=[]
                )
        nc.all_engine_barrier()
        assert self.sems is not None
        popped = nc._tile_sem_poison_stack.pop()
        assert popped is self._sem_poison
        nc.clear_and_free_semaphores(list(self.sems.allocated().values()))
        nc.all_engine_barrier()

    TileContext._drain_and_barrier = patched
    TileContext._drain_cap_patched = True


def _build():
    import concourse.bass as bass
    import concourse.mybir as mybir
    from concourse.tile import TileContext
    from contextlib import ExitStack
    from collections import deque

    _patch_drain_cap()

    f32 = mybir.dt.float32
    bf16 = mybir.dt.bfloat16
    AF = mybir.ActivationFunctionType

    nc = bass.Bass()
    xt_p = nc.declare_dram_parameter("xt", [C, N], bf16, isOutput=False)
    wq_p = nc.declare_dram_parameter("wqi", [128, 2048], bf16, isOutput=False)
    wk_p = nc.declare_dram_parameter("wki", [128, 2048], bf16, isOutput=False)
    wv_p = nc.declare_dram_parameter("wvi", [128, 2048], bf16, isOutput=False)
    wo_p = nc.declare_dram_parameter("woT", [DL, C], bf16, isOutput=False)
    bqk_p = nc.declare_dram_parameter("bqk", [128, 4], f32, isOutput=False)
    out_p = nc.declare_dram_parameter("out", [N, C], bf16, isOutput=True)

    with TileContext(nc) as tc, ExitStack() as ctx:
        # ---- long-lived SBUF pools ----
        wpool = ctx.enter_context(tc.tile_pool(name="w", bufs=1))
        qkpool = ctx.enter_context(tc.tile_pool(name="qk", bufs=1))
        vpool = ctx.enter_context(tc.tile_pool(name="v", bufs=1))
        otpool = ctx.enter_context(tc.tile_pool(name="ot", bufs=1))
        xpool = ctx.enter_context(tc.tile_pool(name="x", bufs=4))
        ptpool = ctx.enter_context(tc.tile_pool(name="pt", bufs=22))
        opool = ctx.enter_context(tc.tile_pool(name="o", bufs=24))
        recpool = ctx.enter_context(tc.tile_pool(name="rec", bufs=20))
        obpool = ctx.enter_context(tc.tile_pool(name="ob", bufs=4))

        # ---- weights / constants into SBUF ----
        wq_sb = wpool.tile([128, 8 * DL], bf16, tag="wq")
        wk_sb = wpool.tile([128, 8 * DL], bf16, tag="wk")
        wv_sb = wpool.tile([128, 8 * DL], bf16, tag="wv")
        wo_sb = wpool.tile([128, 2 * C], bf16, tag="wo")
        bqk_sb = wpool.tile([128, 4], f32, tag="bqk")
        bq_sb = bqk_sb[:, 0:2]
        bk_sb = bqk_sb[:, 2:4]

        # x chunks: one tile per 512-token window, feature-chunk-major
        xts = [xpool.tile([128, 8 * 512], bf16, tag="xt", name=f"xt{i}")
               for i in range(4)]

        # All input DMAs on the gpsimd (Pool) queue in priority order.
        def dma_x(nchi, c0=0, c1=8):
            nc.gpsimd.dma_start(
                out=xts[nchi].rearrange("p (c n) -> p c n", c=8)[:, c0:c1],
                in_=xt_p.rearrange("(c p) n -> p c n", p=128)
                [:, c0:c1, nchi * 512:(nchi + 1) * 512])

        def dma_w(sb, p, pr=None):
            # weights arrive as the exact SBUF image -> contiguous rows
            s = (slice(None) if pr is None
                 else slice(pr * 1024, (pr + 1) * 1024))
            nc.gpsimd.dma_start(out=sb[:, s], in_=p[:, s])

        dma_x(0, 0, 4)
        dma_w(wq_sb, wq_p, 0)
        dma_x(0, 4, 8)
        dma_w(wk_sb, wk_p, 0)
        nc.gpsimd.dma_start(out=bqk_sb[:], in_=bqk_p[:])
        dma_x(1)
        dma_w(wv_sb, wv_p)
        dma_w(wq_sb, wq_p, 1)
        dma_w(wk_sb, wk_p, 1)
        dma_x(2)
        dma_x(3)
        nc.gpsimd.dma_start(out=wo_sb.rearrange("p (c d) -> p c d", c=2),
                            in_=wo_p.rearrange("(c p) d -> p c d", p=128))

        # feature-major Q^T,K^T per head-pair; token-major V with the
        # constant-1.0 column per (block, head)
        QT = [qkpool.tile([128, N], bf16, tag=f"qt{p}", name=f"QT{p}")
              for p in range(2)]
        KT = [qkpool.tile([128, N], bf16, tag=f"kt{p}", name=f"KT{p}")
              for p in range(2)]
        V_sb = vpool.tile([128, NB * DLV], bf16, tag="v")
        nc.vector.memset(
            V_sb.rearrange("p (g e) -> p g e", e=D + 1)[:, :, D:D + 1], 1.0)
        OT = [otpool.tile([128, N], bf16, tag=f"ot{p}", name=f"OT{p}")
              for p in range(2)]

        # ---- PSUM layout (8 banks exactly) ----
        # pss: 2 x [128,1024] f32 (4 banks) - scores / exp ping-pong;
        #      phase-1 QKV accumulation borrows tiles from this pool.
        # po:  2 x [128,512] f32 - 8 AV accumulation regions of 65 cols
        #      (4 per bank; ONE accumulation group per bank: start only
        #      on the bank's first write of a head, stop on its last).
        # pc:  2 x [128,512] f32 - out-projection accumulators; claims
        #      alternate banks so start_tensor_calc never serializes
        #      against the previous unit's copy.
        pss = ctx.enter_context(tc.tile_pool(name="pss", bufs=2, space="PSUM"))
        pop = ctx.enter_context(tc.tile_pool(name="po", bufs=1, space="PSUM"))
        po_t = [pop.tile([128, 512], f32, tag=f"po{i}", name=f"po{i}")
                for i in range(2)]
        pcp = ctx.enter_context(tc.tile_pool(name="pc", bufs=1, space="PSUM"))
        pc_t = [pcp.tile([128, 512], f32, tag=f"pc{i}", name=f"pc{i}")
                for i in range(2)]

        # warm-up: ramp the PE p-state clock with dummy matmuls and
        # preload the Exp activation table while input DMAs stream in.
        warm = wpool.tile([1, 64], bf16, tag="warm")
        nc.vector.memset(warm[:], 0.0)
        nc.scalar.activation(warm[:, 0:1], warm[:, 1:2], AF.Exp)
        for _ in range(80):
            nc.tensor.matmul(pc_t[0][0:64, 0:64], warm[:], warm[:],
                             start=True, stop=True)

        def po_region(qb):
            return po_t[qb // 4][:, (qb % 4) * 65:(qb % 4) * 65 + 65]

        # ---------------- background work queue ----------------
        # Items: (due_unit, ready_unit, cost_ns, emit_fn).
        # due: emission deadline (correctness: must precede its consumer
        #      in program order) -> deadline pass scans the whole queue.
        # ready: earliest unit whose inputs are likely resident (perf
        #      heuristic only; real deps are semaphore-tracked).
        bg = deque()
        state = {"budget": 0.0}

        def pump(unit, budget_ns):
            state["budget"] += budget_ns
            # deadline pass: emit the queue PREFIX up to the last due
            # item (items must stay in push order - accumulation groups
            # of consecutive items share PSUM banks)
            last_due = -1
            for i, it in enumerate(bg):
                if it[0] <= unit:
                    last_due = i
            for _ in range(last_due + 1):
                bg.popleft()[3]()   # deadline-forced: don't charge budget
            while bg and state["budget"] > 0 and bg[0][1] <= unit:
                due, ready, cost, fn = bg.popleft()
                fn()
                state["budget"] -= cost
            if not bg and state["budget"] > 0:
                state["budget"] = 0.0

        # ---------------- phase-1 micro-item emitters ----------------
        # Phase-1 accumulates in the pc banks (free until the first
        # out-projection at unit ~64) and is emitted in ~0.2-0.4us
        # micro-items so a pumped item never exceeds a unit's slack.
        # Items alternate pc banks; one accumulation group per bank at
        # a time (micros of one item are pumped in FIFO order).
        ph1 = {"bank": 0}

        def qk_micros(pr, nchi, which, split_copy=False):
            w_sb, bias, dst = ((wq_sb, bq_sb, QT) if which == "q"
                               else (wk_sb, bk_sb, KT))
            acc = pc_t[ph1["bank"]][:, 0:512]
            ph1["bank"] ^= 1

            def mm(ccp):
                def fn():
                    for cc in (2 * ccp, 2 * ccp + 1):
                        nc.tensor.matmul(
                            acc,
                            w_sb[:, pr * 1024 + cc * 128:pr * 1024 + (cc + 1) * 128],
                            xts[nchi][:, cc * 512:(cc + 1) * 512],
                            start=(cc == 0), stop=(cc == 7))
                return fn

            def cp(c0=0, c1=512):
                def fn():
                    nc.vector.tensor_scalar_add(
                        dst[pr][:, nchi * 512 + c0:nchi * 512 + c1],
                        acc[:, c0:c1], bias[:, pr:pr + 1])
                return fn
            if split_copy:
                # first 128 cols first: the first scores' stationary
                # operand only needs k-block 0
                return [(440, mm(0)), (440, mm(1)), (440, mm(2)),
                        (440, mm(3)), (280, cp(0, 128)), (560, cp(128, 512))]
            return [(440, mm(0)), (440, mm(1)), (440, mm(2)), (440, mm(3)),
                    (680, cp())]

        def v_micros(nchi, b):
            """One 128-token V block (all 4 heads + bias)."""
            nb = nchi * 4 + b
            acc = pc_t[ph1["bank"]][:, 0:256]
            ph1["bank"] ^= 1

            def mm(c0, c1):
                def fn():
                    for cc in range(c0, c1):
                        nc.tensor.matmul(
                            acc,
                            xts[nchi][:, cc * 512 + b * 128:
                                      cc * 512 + (b + 1) * 128],
                            wv_sb[:, cc * DL:(cc + 1) * DL],
                            start=(cc == 0), stop=(cc == 7))
                return fn

            def cp():
                nc.vector.tensor_copy(
                    V_sb[:, nb * DLV:(nb + 1) * DLV]
                    .rearrange("p (h e) -> p h e", h=HPC)[:, :, 0:D],
                    acc.rearrange("p (h e) -> p h e", h=HPC))
            return [(330, mm(0, 3)), (330, mm(3, 6)), (360, mm(6, 8)),
                    (420, cp)]

        def run_item(micros):
            for _, fn in micros:
                fn()

        vst = {"done": 0}

        def push_item(due, ready, micros, is_v=False):
            n = len(micros)
            for i, (cost, fn) in enumerate(micros):
                if is_v and i == n - 1:
                    def wrap(f=fn):
                        f()
                        vst["done"] += 1
                    bg.append((due, ready, cost, wrap))
                else:
                    bg.append((due, ready, cost, fn))

        # ---------------- phase-2 helpers ----------------
        pair_tiles = {}

        def transpose_item(o_t, pr, qh, qb):
            def fn():
                nc.sync.dma_start(
                    out=OT[pr][:, qh * 1024 + qb * 128:
                               qh * 1024 + (qb + 1) * 128],
                    in_=o_t[:], transpose=True)
            return fn

        def norms(qh, h):
            """Normalize the 8 completed q-blocks of head h into the
            head-pair O tiles; after the odd head, queue the pair's
            DMA transposes."""
            pr, off = h // 2, 64 * (h % 2)
            for qb in range(8):
                reg = po_region(qb)
                rec = recpool.tile([128, 1], f32, tag="rec",
                                   name=f"rec{qh}{h}{qb}")
                nc.vector.reciprocal(rec[:], reg[:, D:D + 1])
                if h % 2 == 0:
                    pair_tiles[qb] = opool.tile([128, 2 * D], bf16, tag="o",
                                                name=f"o{qh}{h}{qb}")
                o_t = pair_tiles[qb]
                nc.vector.tensor_scalar_mul(
                    o_t[:, off:off + D], reg[:, 0:D], rec[:])
                if h % 2 == 1:
                    bg.append((10 ** 9, 0, 60,
                               transpose_item(o_t, pr, qh, qb)))

        ob_ctr = {"i": 0, "mode": "pc"}

        def op_region():
            if ob_ctr["mode"] == "pc":
                regs = [pc_t[0][:, 0:256], pc_t[1][:, 0:256],
                        pc_t[0][:, 256:512], pc_t[1][:, 256:512]]
            else:   # tail: po banks are free too - rotate over 4 banks
                regs = [pc_t[0][:, 0:256], pc_t[1][:, 0:256],
                        po_t[0][:, 0:256], po_t[1][:, 0:256],
                        pc_t[0][:, 256:512], pc_t[1][:, 256:512],
                        po_t[0][:, 256:512], po_t[1][:, 256:512]]
            r = regs[ob_ctr["i"] % len(regs)]
            ob_ctr["i"] += 1
            return r

        ob_nb = {}

        def outproj_item(qh, nb, cchalf, copy_eng):
            def fn():
                acc = op_region()
                c0 = cchalf * 256
                for pr in range(2):
                    nc.tensor.matmul(
                        acc, OT[pr][:, nb * 128:(nb + 1) * 128],
                        wo_sb[:, pr * C + c0:pr * C + c0 + 256],
                        start=(pr == 0), stop=(pr == 1))
                if cchalf == 0:
                    ob_nb[nb] = obpool.tile([128, C], bf16, tag="ob",
                                            name=f"ob{qh}{nb}")
                copy_eng(ob_nb[nb][:, c0:c0 + 256], acc)
                if qh == 1 and nb >= 14:
                    # last blocks: split the store so the final transfer
                    # is half-sized and both queues finish early
                    if cchalf == 1:
                        nc.sync.dma_start(
                            out=out_p[nb * 128:(nb + 1) * 128, 0:512],
                            in_=ob_nb[nb][:, 0:512])
                    elif cchalf == 3:
                        nc.gpsimd.dma_start(
                            out=out_p[nb * 128:(nb + 1) * 128, 512:1024],
                            in_=ob_nb[nb][:, 512:1024])
                elif cchalf == 3:
                    eng = nc.gpsimd if nb % 2 else nc.sync
                    eng.dma_start(out=out_p[nb * 128:(nb + 1) * 128, :],
                                  in_=ob_nb[nb][:])
            return fn

        def dve_copy(dst, src):
            nc.vector.tensor_copy(dst, src)

        def act_copy(dst, src):
            nc.scalar.activation(dst, src, AF.Identity)

        # ---------------- prefix ----------------
        run_item(qk_micros(0, 0, "q"))
        run_item(qk_micros(0, 0, "k", split_copy=True))

        # remaining phase-1 work with due-units.  V blocks are consumed
        # via the dynamic AV gate (v_done), so their dues are only a
        # backstop; KT dues are hard scores deadlines.
        push_item(1, 0, v_micros(0, 0), is_v=True)
        push_item(3, 0, qk_micros(0, 1, "k"))
        push_item(6, 0, v_micros(0, 1), is_v=True)
        push_item(7, 2, qk_micros(0, 2, "k"))
        push_item(8, 0, v_micros(0, 2), is_v=True)
        push_item(10, 0, v_micros(0, 3), is_v=True)
        push_item(11, 5, qk_micros(0, 3, "k"))
        push_item(12, 1, v_micros(1, 0), is_v=True)
        push_item(13, 1, v_micros(1, 1), is_v=True)
        push_item(14, 1, v_micros(1, 2), is_v=True)
        push_item(15, 2, v_micros(1, 3), is_v=True)
        push_item(16, 3, v_micros(2, 0), is_v=True)
        push_item(17, 3, v_micros(2, 1), is_v=True)
        push_item(18, 3, v_micros(2, 2), is_v=True)
        push_item(19, 5, v_micros(2, 3), is_v=True)
        push_item(20, 5, v_micros(3, 0), is_v=True)
        push_item(21, 5, v_micros(3, 1), is_v=True)
        push_item(22, 5, v_micros(3, 2), is_v=True)
        push_item(23, 5, v_micros(3, 3), is_v=True)
        push_item(25, 6, qk_micros(1, 0, "q"))
        push_item(27, 6, qk_micros(1, 0, "k"))
        push_item(29, 6, qk_micros(1, 1, "q"))
        push_item(31, 6, qk_micros(1, 1, "k"))
        push_item(36, 8, qk_micros(1, 2, "k"))
        push_item(40, 8, qk_micros(1, 3, "k"))
        push_item(62, 8, qk_micros(0, 2, "q"))
        push_item(63, 8, qk_micros(0, 3, "q"))
        push_item(93, 10, qk_micros(1, 2, "q"))
        push_item(95, 10, qk_micros(1, 3, "q"))

        # ---------------- main unit loop ----------------
        # unit u = (qh, h, kb); AV of unit u-1 emitted during unit u
        # (software pipeline).  One PSUM accumulation group per po bank
        # per head: start only on the bank's first matmul (kb0, qb 0/4),
        # stop on its last (kb15, qb 3/7); intermediate writes land on
        # pending-zero bytes and replace, which zeroes implicitly.
        def emit_av(pt, kb, h, start, stop):
            base = kb * DLV + h * (D + 1)
            for qb in range(8):
                nc.tensor.matmul(
                    po_region(qb),
                    pt[:, qb * 128:(qb + 1) * 128],
                    V_sb[:, base:base + D + 1],
                    start=start and qb % 4 == 0,
                    stop=stop and qb % 4 == 3)

        def push_outproj(qh):
            for nb in range(qh * 8, qh * 8 + 8):
                for cq in range(4):
                    eng = dve_copy if qh == 0 else (
                        dve_copy if cq % 2 == 0 else act_copy)
                    bg.append((10 ** 9, 0, 640, outproj_item(qh, nb, cq, eng)))

        def finish_head(p_qh, p_h):
            norms(p_qh, p_h)
            if p_h == HPC - 1:
                push_outproj(p_qh)

        av_q = deque()       # (pt, kb, qh, h, unit) gated AV emissions
        unit = 0
        for qh in range(2):
            for h in range(HPC):
                pr, off = h // 2, 64 * (h % 2)
                for kb in range(NB):
                    pump(unit, 0)   # overdue items first
                    ps = pss.tile([128, 1024], f32, tag="ps",
                                  name=f"ps{qh}{h}{kb}")
                    pt = ptpool.tile([128, 1024], bf16, tag="pt",
                                     name=f"pt{qh}{h}{kb}")
                    for j in range(2):
                        if unit == 0 and j == 1:
                            # q-cols 512:1024 need the second x-chunk;
                            # emit their projection only now so the j=0
                            # half-exp above wasn't stuck behind it
                            run_item(qk_micros(0, 1, "q"))
                        nc.tensor.matmul(
                            ps[:, j * 512:(j + 1) * 512],
                            KT[pr][off:off + 64, kb * 128:(kb + 1) * 128],
                            QT[pr][off:off + 64,
                                   qh * 1024 + j * 512:qh * 1024 + (j + 1) * 512],
                            start=True, stop=True)
                        if unit == 0:
                            # first unit: exp per half so the exp stream
                            # starts before the second x-chunk lands
                            nc.scalar.activation(
                                pt[:, j * 512:(j + 1) * 512],
                                ps[:, j * 512:(j + 1) * 512], AF.Exp)
                    if unit > 0:
                        nc.scalar.activation(pt[:], ps[:], AF.Exp)
                    av_q.append((pt, kb, qh, h, unit))
                    n_av = 0
                    while av_q and n_av < 3:
                        a_pt, a_kb, a_qh, a_h, a_u = av_q[0]
                        if a_u >= unit or a_kb >= vst["done"]:
                            break
                        av_q.popleft()
                        emit_av(a_pt, a_kb, a_h, a_kb == 0, a_kb == NB - 1)
                        n_av += 1
                        if a_kb == NB - 1:
                            finish_head(a_qh, a_h)
                            break   # let norms land before the next claim
                    pump(unit, 420)
                    unit += 1
        # ---- tail: drain the gated AV queue, then an interleaved
        # norm / transpose / out-projection pipeline for the last head,
        # with copies split across DVE and the now-idle ScalarE and the
        # out-projection accumulators rotating over the free PSUM banks.
        p_qh, p_h = 1, HPC - 1
        while av_q:
            a_pt, a_kb, a_qh, a_h, a_u = av_q.popleft()
            emit_av(a_pt, a_kb, a_h, a_kb == 0, a_kb == NB - 1)
            if a_kb == NB - 1 and not (a_qh == p_qh and a_h == p_h):
                finish_head(a_qh, a_h)
        pump(10 ** 9, 10 ** 9)      # drain leftover background work
        pr, off = p_h // 2, 64 * (p_h % 2)
        for qb in range(8):
            reg = po_region(qb)
            rec = recpool.tile([128, 1], f32, tag="rec", name=f"rect{qb}")
            nc.vector.reciprocal(rec[:], reg[:, D:D + 1])
            nc.vector.tensor_scalar_mul(
                pair_tiles[qb][:, off:off + D], reg[:, 0:D], rec[:])
            nc.sync.dma_start(
                out=OT[pr][:, 1024 + qb * 128:1024 + (qb + 1) * 128],
                in_=pair_tiles[qb][:], transpose=True)
        # only after every po region has been read may the out-projection
        # accumulators rotate into the po banks
        ob_ctr["mode"] = "tail"
        for qb in range(8):
            for cq in range(4):
                eng = dve_copy if cq % 2 == 0 else act_copy
                outproj_item(1, 8 + qb, cq, eng)()
    return nc


def _w_img_qk(wT):
    # [1024, 256] -> SBUF image [128, 2048], col = pr*1024 + cc*128 + d
    bf = ml_dtypes.bfloat16
    return np.ascontiguousarray(
        wT.reshape(8, 128, 2, 128).transpose(1, 2, 0, 3).reshape(128, 2048)
    ).astype(bf)


def _w_img_v(wT):
    # [1024, 256] -> SBUF image [128, 2048], col = cc*256 + d
    bf = ml_dtypes.bfloat16
    return np.ascontiguousarray(
        wT.reshape(8, 128, 256).transpose(1, 0, 2).reshape(128, 2048)
    ).astype(bf)


def _prep_in_maps(x, qkv_w, qkv_b, out_w):
    bf = ml_dtypes.bfloat16
    in_maps = []
    for c in range(NCORES):
        b, hg = c // 4, c % 4
        h0 = 4 * hg
        qsl = slice(h0 * D, (h0 + 4) * D)
        ksl = slice(C + h0 * D, C + (h0 + 4) * D)
        vsl = slice(2 * C + h0 * D, 2 * C + (h0 + 4) * D)
        in_maps.append({
            "xt": np.ascontiguousarray(x[b].T).astype(bf),
            "wqi": _w_img_qk(qkv_w[qsl].T * 0.125),
            "wki": _w_img_qk(qkv_w[ksl].T),
            "wvi": _w_img_v(qkv_w[vsl].T),
            "woT": np.ascontiguousarray(out_w[:, h0 * D:(h0 + 4) * D].T).astype(bf),
            "bqk": np.ascontiguousarray(np.concatenate([
                (qkv_b[qsl] * 0.125).reshape(2, 128).T,
                qkv_b[ksl].reshape(2, 128).T], axis=1)).astype(np.float32),
        })
    return in_maps


def kernel(x, qkv_w, qkv_b, out_w, out_b):
    from concourse.bass_utils import run_bass_kernel_spmd

    x = np.asarray(x, dtype=np.float32)
    qkv_w = np.asarray(qkv_w, dtype=np.float32)
    qkv_b = np.asarray(qkv_b, dtype=np.float32)
    out_w = np.asarray(out_w, dtype=np.float32)
    out_b = np.asarray(out_b, dtype=np.float32)

    if "nc" not in _cache:
        _cache["nc"] = _build()
    in_maps = _prep_in_maps(x, qkv_w, qkv_b, out_w)
    res = run_bass_kernel_spmd(_cache["nc"], in_maps, list(range(NCORES)))
    out = np.zeros((B, N, C), np.float32)
    for c in range(NCORES):
        out[c // 4] += res.results[c]["out"].astype(np.float32)
    out += (out_b + qkv_b[2 * C:] @ out_w.T)[None, None, :]
    return out
